# revision 27
# baseline (speedup 1.0000x reference)
"""Trainium2 Bass kernel for the HMM forward-algorithm problem.

Strategy
--------
The reference does, per time step, a log-domain matrix-vector product
  alpha_t[b,k] = em[b,t,k] + logsumexp_j(alpha_{t-1}[b,j] + tran[j,k])
followed by logsumexp_k.  We run the whole recurrence in *probability*
domain on the TensorEngine:

  phat_t = E_t  *  (phat_{t-1} @ P)          (elementwise * matmul)

where P = softmax(tran) rows (constant) and E_t = exp(em_t - kappa) with a
global shift kappa that keeps E <= ~1.  The per-step logsumexp_k output
reduces to log(sum_k phat_t) + known offsets; column sums are accumulated
on the TensorEngine with a ones-vector matmul into a per-block PSUM strip.
phat decays by ~e^-3 per step, so every RN=16 steps it is rescaled by the
bf16 reciprocal of a *stale* (4-step-old) column sum — the scale is
recorded and undone exactly on the host, and using a stale value keeps the
reciprocal chain off the critical path.

The serial per-step chain (PE matmuls -> sem -> DVE multiply -> sem -> PE)
is ~590ns of mostly pipeline-drain/semaphore/PSUM-access latency and is the
throughput wall; the 8 batch rows per core run as TWO groups of 4 so the
smaller DVE multiply (142ns vs 158ns) shortens that chain, with all other
work (emissions, column sums, renorm plumbing) scheduled into its idle
windows.

Emissions: em[b,t,h] = 0.25 * sum_s x[s,h,obs[b,t,s]] - L[h].  The host
pre-transposes x to a (S*V, H) bf16 row table; the device gathers rows
with indirect DMA (128 rows = 16 timesteps x 8 batch) one source at a
time, then sums the 4 sources AND transposes to H-major in one shot using
PSUM-accumulating identity matmuls on the TensorEngine, and applies
exp(0.25*x - L - kappa) on the ScalarEngine into the E-strip consumed by
the scan.  Block 0's E-strip is precomputed on the host to skip the
startup gather latency.

Sharding: data-parallel over batch (8 of 64 rows per core).  Tables are
replicated.  No collectives.
"""
import sys

sys.path.insert(0, "/opt/trn_rl_repo")

import numpy as np
import ml_dtypes

import concourse.bass as bass
import concourse.bacc as bacc
import concourse.tile as tile
import concourse.mybir as mybir
import concourse.bass_utils as bass_utils
from concourse.masks import make_identity

B, T, S, H, V = 64, 512, 4, 512, 10000
NC = 8            # cores
BL = B // NC      # batch rows per core
G = 2             # staggered scan groups per core
BG = BL // G      # batch rows per group
P_ = 128          # partitions
HCN = H // P_     # h chunks
TBLK = 16         # timesteps per gather block
RN = 16           # renorm interval
STALE = 4         # renorm uses colsum of phat_{t-STALE}
F32 = mybir.dt.float32
BF16 = mybir.dt.bfloat16
I32 = mybir.dt.int32
EXP = mybir.ActivationFunctionType.Exp
MULT = mybir.AluOpType.mult

_compiled = {}
LAST_T = T


def _renorm_steps(t_steps):
    return [t for t in range(1, t_steps) if t % RN == 0]


def build(t_steps=T):
    """Build + bacc-compile the per-core Bass program (identical on all cores)."""
    nblk = -(-t_steps // TBLK)
    renorms = _renorm_steps(t_steps)
    nrn = max(1, len(renorms))
    nc = bacc.Bacc("TRN2", target_bir_lowering=False, debug=False,
                   enable_asserts=False, num_devices=NC)

    tabt = nc.dram_tensor("tabt", [S * V, H], BF16, kind="ExternalInput").ap()
    pm_d = nc.dram_tensor("pm", [P_, HCN * HCN * P_], BF16, kind="ExternalInput").ap()
    idx_d = nc.dram_tensor("idx", [P_, S * nblk], I32, kind="ExternalInput").ap()
    bias_d = nc.dram_tensor("bias", [P_, HCN], F32, kind="ExternalInput").ap()
    eb0_d = nc.dram_tensor("eb0", [P_, 2 * TBLK * HCN * BL], BF16,
                           kind="ExternalInput").ap()
    rstrip_d = nc.dram_tensor("rstrip", [1, nblk * TBLK * BL], F32,
                              kind="ExternalOutput").ap()
    rinv_d = nc.dram_tensor("rinvstrip", [1, nrn * BL], F32,
                            kind="ExternalOutput").ap()

    with tile.TileContext(nc) as tc:
        with (tc.tile_pool(name="const", bufs=1) as cp,
              tc.tile_pool(name="estrip", bufs=3) as ep,
              tc.tile_pool(name="gath", bufs=10) as gp,
              tc.tile_pool(name="phat", bufs=3 * G) as pp,
              tc.tile_pool(name="small", bufs=4) as sp,
              tc.tile_pool(name="qpsum", bufs=1, space="PSUM") as qp,
              tc.tile_pool(name="rspsum", bufs=2, space="PSUM") as rsp,
              tc.tile_pool(name="tpsum", bufs=2, space="PSUM") as tp_,
              tc.tile_pool(name="ipsum", bufs=1, space="PSUM") as ip):

            # ---- constants (eb0+pm first, in parallel on two queues:
            # they gate the first scan step; idx/bias only gate gathers) ----
            pm_t = cp.tile([P_, HCN * HCN * P_], BF16, name="pmt")
            nc.scalar.dma_start(pm_t[:, :], pm_d[:, :])
            idx_t = cp.tile([P_, S * nblk], I32, name="idxt")
            nc.sync.dma_start(idx_t[:, :], idx_d[:, :])
            bias_t = cp.tile([P_, HCN], F32, name="biast")
            nc.scalar.dma_start(bias_t[:, :], bias_d[:, :])
            eb0_t = cp.tile([P_, 2 * TBLK * HCN * BL], BF16, name="eb0t")
            nc.sync.dma_start(eb0_t[:, :], eb0_d[:, :])
            ones128 = cp.tile([P_, 1], BF16, name="ones128")
            nc.gpsimd.memset(ones128[:, :], 1.0)
            onesrow = cp.tile([1, P_], BF16, name="onesrow")
            nc.gpsimd.memset(onesrow[:, :], 1.0)
            ident = cp.tile([P_, P_], BF16, name="ident")
            make_identity(nc, ident[:, :])
            rstrip_t = cp.tile([1, nblk * TBLK * BL], F32, name="rstript")
            rinv_t = cp.tile([1, nrn * BL], F32, name="rinvt")

            eb_list = [None] * nblk
            eb_list[0] = eb0_t[:, :TBLK * HCN * BL]
            if nblk > 1:
                eb_list[1] = eb0_t[:, TBLK * HCN * BL:]

            g_tiles = {}     # blk -> [4 gather tiles]
            tp_tiles = {}    # (blk, c) -> transpose psum tile
            rs_tiles = {}    # blk -> rstrip psum tile
            rinv_sb = {}     # (t, g) -> bf16 reciprocal staging tile
            rinv_ps = {}     # (t, g) -> [128,16] broadcast psum tile

            def emit_gather(blk, s):
                g = gp.tile([P_, H], BF16, tag="g", name=f"g{blk}_{s}")
                col = s * nblk + blk
                nc.gpsimd.indirect_dma_start(
                    out=g[:, :], out_offset=None, in_=tabt[:, :],
                    in_offset=bass.IndirectOffsetOnAxis(
                        ap=idx_t[:, col:col + 1], axis=0))
                g_tiles.setdefault(blk, [None] * S)[s] = g

            def emit_transpose(blk, c, s):
                if s == 0:
                    tp_tiles[(blk, c)] = tp_.tile([P_, P_], F32, tag="tp", name=f"tp{blk}_{c}")
                nc.tensor.matmul(tp_tiles[(blk, c)][:, :],
                                 lhsT=g_tiles[blk][s][:, c * P_:(c + 1) * P_],
                                 rhs=ident[:, :],
                                 start=(s == 0), stop=(s == S - 1))

            def emit_act(blk, c):
                if c == 0:
                    eb_list[blk] = ep.tile([P_, TBLK * HCN * BL], BF16,
                                           tag="eb", name=f"eb{blk}")
                eb4 = eb_list[blk].rearrange("p (t c b) -> p t c b",
                                             t=TBLK, c=HCN)
                tpp = tp_tiles.pop((blk, c))
                nc.scalar.activation(
                    eb4[:, :, c, :],
                    tpp.rearrange("p (t b) -> p t b", t=TBLK),
                    EXP, bias=bias_t[:, c:c + 1], scale=0.25)

            def rs_tile(m):
                if m not in rs_tiles:
                    rs_tiles[m] = rsp.tile([1, TBLK * BL], F32, tag="rs", name=f"rs{m}")
                return rs_tiles[m]

            def emit_rgroup(t_src, phat_jc, g):
                """Column sums of phat_{t_src} (group g) -> psum slot t_src."""
                rt = rs_tile(t_src // TBLK)
                slot = t_src % TBLK
                cs = slice(slot * BL + g * BG, slot * BL + (g + 1) * BG)
                for jc in range(HCN):
                    nc.tensor.matmul(rt[:, cs], lhsT=ones128[:, :],
                                     rhs=phat_jc[jc],
                                     start=(jc == 0), stop=(jc == HCN - 1))

            def emit_rs_copy(m):
                """PSUM rstrip block -> SBUF strip -> DRAM (streamed)."""
                rt = rs_tiles.pop(m)
                cs = slice(m * TBLK * BL, (m + 1) * TBLK * BL)
                nc.scalar.copy(rstrip_t[:, cs], rt[:, :])
                nc.sync.dma_start(rstrip_d[:, cs], rstrip_t[:, cs])

            # blocks 0 and 1 come precomputed from the host; prefetch
            # gathers for blocks 2 and 3 before the scan starts
            for m in (2, 3):
                if m < nblk:
                    for s in range(S):
                        emit_gather(m, s)

            eb0_4 = eb_list[0].rearrange("p (t c b) -> p t c b",
                                         t=TBLK, c=HCN)
            # phat[g] is a list of per-jc [128, BG] APs (matmul rhs slices)
            phat = [[eb0_4[:, 0, jc, g * BG:(g + 1) * BG]
                     for jc in range(HCN)] for g in range(G)]

            ridx_of = {t: i for i, t in enumerate(renorms)}

            for t in range(1, t_steps):
                blk, ph = t // TBLK, t % TBLK

                # ---- emission pipeline bookkeeping for this phase ----
                if ph % (TBLK // S) == 1 and blk >= 1 and blk + 3 < nblk:
                    emit_gather(blk + 3, (ph - 1) // (TBLK // S))
                if blk >= 1 and blk + 1 < nblk:
                    c, s = ph // S, ph % S
                    emit_transpose(blk + 1, c, s)
                    if s == S - 1:
                        emit_act(blk + 1, c)

                # ---- scan step: both groups' matmul bundles first ----
                qs = []
                for g in range(G):
                    q = qp.tile([P_, HCN * BG], F32, tag=f"q{g}", name=f"q{t}_{g}")
                    for kc in range(HCN):
                        for jc in range(HCN):
                            nc.tensor.matmul(
                                q[:, kc * BG:(kc + 1) * BG],
                                lhsT=pm_t[:, (jc * HCN + kc) * P_:
                                          (jc * HCN + kc + 1) * P_],
                                rhs=phat[g][jc],
                                start=(jc == 0), stop=(jc == HCN - 1))
                    qs.append(q)
                if t + 1 in ridx_of:
                    # broadcast bf16 rinv (from stale colsum) to [128,16];
                    # applied to the E-strip slice of step t+1 below
                    for g in range(G):
                        rps = ip.tile([P_, HCN * BG], F32, tag=f"rp{g}",
                                      name=f"rp{t}_{g}")
                        rb = rinv_sb[(t + 1, g)]
                        for c in range(HCN):
                            nc.tensor.matmul(rps[:, c * BG:(c + 1) * BG],
                                             lhsT=onesrow[:, :],
                                             rhs=rb[:, :],
                                             start=True, stop=True)
                        rinv_ps[(t + 1, g)] = rps
                # column sums of the previous step's phat (same dep as mms,
                # ordered after them so they don't delay the q semaphores)
                for g in range(G):
                    emit_rgroup(t - 1, phat[g], g)

                for g in range(G):
                    tn2 = t + 2
                    if tn2 in ridx_of and tn2 < t_steps:
                        # colsum of phat_{tn2-STALE} lives in slot tn2-STALE
                        src_t = rs_tile((tn2 - STALE) // TBLK)
                        slot = (tn2 - STALE) % TBLK
                        cs = slice(slot * BL + g * BG,
                                   slot * BL + (g + 1) * BG)
                        rb = sp.tile([1, BG], BF16, tag=f"rb{g}", name=f"rb{t}_{g}")
                        with nc.allow_low_precision(
                                reason="renorm scale recorded+undone exactly"):
                            nc.vector.reciprocal(rb[:, :], src_t[:, cs])
                        rinv_sb[(tn2, g)] = rb
                        ri = ridx_of[tn2]
                        nc.scalar.copy(
                            rinv_t[:, ri * BL + g * BG:
                                   ri * BL + (g + 1) * BG], rb[:, :])

                ebt4 = eb_list[blk].rearrange("p (t c b) -> p t c b",
                                              t=TBLK, c=HCN)
                for g in range(G):
                    pnew = pp.tile([P_, HCN * BG], BF16, tag=f"ph{g}", name=f"pn{t}_{g}")
                    pnew3 = pnew.rearrange("p (c b) -> p c b", c=HCN)
                    q3 = qs[g].rearrange("p (c b) -> p c b", c=HCN)
                    nc.vector.tensor_tensor(
                        pnew3[:, :, :], q3[:, :, :],
                        ebt4[:, ph, :, g * BG:(g + 1) * BG], MULT)
                    phat[g] = [pnew[:, jc * BG:(jc + 1) * BG]
                               for jc in range(HCN)]

                # off-critical-path renorm plumbing (stale by >= 2 steps)
                for g in range(G):
                    tn = t + 1
                    if tn in ridx_of and tn < t_steps:
                        # pre-scale step t+1's E-strip slice by the stale
                        # reciprocal (SBUF-only multiply, off the scan loop)
                        rps = rinv_ps.pop((tn, g))
                        rps3 = rps.rearrange("p (c b) -> p c b", c=HCN)
                        nblk4 = eb_list[tn // TBLK].rearrange(
                            "p (t c b) -> p t c b", t=TBLK, c=HCN)
                        nc.vector.tensor_tensor(
                            nblk4[:, tn % TBLK, :, g * BG:(g + 1) * BG],
                            nblk4[:, tn % TBLK, :, g * BG:(g + 1) * BG],
                            rps3[:, :, :], MULT)

                # block m's psum strip complete once slot 15 written (t-1)
                if ph == 0 and blk >= 1:
                    emit_rs_copy(blk - 1)

            # ---- tail: colsums of the final phat, last block copy, DMA out
            for g in range(G):
                emit_rgroup(t_steps - 1, phat[g], g)
            emit_rs_copy(t_steps // TBLK - 1 if t_steps % TBLK == 0
                         else t_steps // TBLK)
            nc.sync.dma_start(rinv_d[:, :], rinv_t[:, :])

    nc.compile()
    return nc


def _get_compiled(t_steps=T):
    if t_steps not in _compiled:
        _compiled[t_steps] = build(t_steps)
    return _compiled[t_steps]


def _host_prep(obs, emis, tran, priors, t_steps):
    """Returns (shared_inputs, per_core_inputs, kappa)."""
    nblk = -(-t_steps // TBLK)
    tpad = nblk * TBLK
    if tpad > obs.shape[1]:
        obs = np.concatenate(
            [obs, np.repeat(obs[:, -1:, :], tpad - obs.shape[1], axis=1)],
            axis=1)
    # transition softmax -> bf16 chunk layout [j, (jc*HCN+kc)*128 + k]
    m = tran.max(axis=1, keepdims=True)
    e = np.exp(tran - m, dtype=np.float32)
    P = (e / e.sum(axis=1, keepdims=True)).astype(ml_dtypes.bfloat16)
    pm = np.ascontiguousarray(
        P.reshape(HCN, P_, HCN, P_).transpose(1, 0, 2, 3).reshape(P_, -1))

    # transposed bf16 emission table, rows indexed by s*V+v
    tabT = np.ascontiguousarray(
        emis.transpose(0, 2, 1)).astype(ml_dtypes.bfloat16).reshape(S * V, H)

    # L[h] and kappa
    mx = emis.max(axis=2)                                   # (S,H)
    lse = mx + np.log(np.exp(emis - mx[:, :, None],
                             dtype=np.float32).sum(axis=2))
    L = 0.25 * lse.sum(axis=0)                              # (H,)
    kap_h = 0.25 * mx.sum(axis=0) - L
    kappa = float(kap_h.max())
    bias = np.ascontiguousarray(
        (-(L + kappa)).astype(np.float32).reshape(HCN, P_).T)   # (128,4)
    expp = np.exp(priors, dtype=np.float32)                     # (H,)

    # per-core gather row indices: idx[p=(tt*BL+bb), s*nblk+blk]
    per_core = []
    svec = (np.arange(S, dtype=np.int64) * V)
    tabT_f = tabT.astype(np.float32).reshape(S, V, H)
    for c in range(NC):
        o = obs[c * BL:(c + 1) * BL, :tpad, :]              # (BL,t,S)
        oi = o + svec[None, None, :]
        oi = oi.transpose(1, 0, 2)                          # (t, BL, S)
        oi = oi.reshape(nblk, TBLK, BL, S)
        oi = oi.transpose(1, 2, 3, 0).reshape(TBLK * BL, S * nblk)
        idx = np.ascontiguousarray(oi.astype(np.int32))

        # host-computed E-strip for blocks 0-1: layout [128, (blk, t, c, b)]
        o0 = o[:, :2 * TBLK, :]                             # (BL,32,S)
        x0 = tabT_f[np.arange(S)[None, None, :], o0].sum(axis=2)  # (BL,32,H)
        e0 = np.exp(0.25 * x0 - (L + kappa)[None, None, :])       # (BL,32,H)
        e0[:, 0, :] *= expp[None, :]
        parts = []
        for m in range(2):
            eb = e0[:, m * TBLK:(m + 1) * TBLK].transpose(2, 1, 0)  # (H,16,BL)
            eb = eb.reshape(HCN, P_, TBLK, BL).transpose(1, 2, 0, 3)
            parts.append(eb.reshape(P_, TBLK * HCN * BL))
        eb0 = np.ascontiguousarray(
            np.concatenate(parts, axis=1)).astype(ml_dtypes.bfloat16)
        per_core.append({"idx": idx, "eb0": eb0})

    shared = {"tabt": tabT, "pm": pm, "bias": bias}
    return shared, per_core, kappa


def _host_post(results, lengths, kappa, t_steps):
    renorms = _renorm_steps(t_steps)
    nrn = max(1, len(renorms))
    ans = np.zeros((B, 1), np.float32)
    tt = np.arange(t_steps, dtype=np.float64)
    for c in range(NC):
        r = np.asarray(results[c]["rstrip"]).reshape(-1, BL)[
            :t_steps].astype(np.float64)
        rinv = np.asarray(results[c]["rinvstrip"]).astype(
            np.float64).reshape(nrn, BL)
        rho_log = np.zeros((t_steps, BL), np.float64)
        for k, t in enumerate(renorms):
            rho_log[t] = np.log(rinv[k])
        logsums = np.log(r) + (tt[:, None] + 1.0) * kappa \
            - np.cumsum(rho_log, axis=0)
        lens = np.clip(lengths[c * BL:(c + 1) * BL], 1, t_steps)
        ans[c * BL:(c + 1) * BL, 0] = logsums[
            lens - 1, np.arange(BL)].astype(np.float32)
    return ans


def run(inputs, t_steps=None, trace=False):
    obs = np.asarray(inputs["obs"])
    lengths = np.asarray(inputs["lengths"])
    emis = np.asarray(inputs["unnormalized_emis"], np.float32)
    tran = np.asarray(inputs["unnormalized_tran"], np.float32)
    priors = np.asarray(inputs["log_state_priors"], np.float32)
    if t_steps is None:
        # the scan only needs to reach the longest sequence
        t_steps = int(min(T, max(48, int(lengths.max()))))
    global LAST_T
    LAST_T = t_steps

    nc = _get_compiled(t_steps)
    shared, per_core, kappa = _host_prep(obs, emis, tran, priors, t_steps)
    in_maps = [dict(shared, **per_core[c]) for c in range(NC)]
    res = bass_utils.run_bass_kernel_spmd(nc, in_maps,
                                          core_ids=list(range(NC)),
                                          trace=trace)
    ans = _host_post(res.results, lengths, kappa, t_steps)
    return ans, res


def kernel(obs, lengths, unnormalized_emis, unnormalized_tran,
           log_state_priors):
    ans, _ = run(dict(obs=obs, lengths=lengths,
                      unnormalized_emis=unnormalized_emis,
                      unnormalized_tran=unnormalized_tran,
                      log_state_priors=log_state_priors))
    return ans


# revision 28
# speedup vs baseline: 1.0187x; 1.0187x over previous
"""Trainium2 Bass kernel for the HMM forward-algorithm problem.

Strategy
--------
The reference does, per time step, a log-domain matrix-vector product
  alpha_t[b,k] = em[b,t,k] + logsumexp_j(alpha_{t-1}[b,j] + tran[j,k])
followed by logsumexp_k.  We run the whole recurrence in *probability*
domain on the TensorEngine:

  phat_t = E_t  *  (phat_{t-1} @ P)          (elementwise * matmul)

where P = softmax(tran) rows (constant) and E_t = exp(em_t - kappa) with a
global shift kappa that keeps E <= ~1.  The per-step logsumexp_k output
reduces to log(sum_k phat_t) + known offsets; column sums are accumulated
on the TensorEngine with a ones-vector matmul into a per-block PSUM strip.
phat decays by ~e^-3 per step, so every RN=16 steps it is rescaled by the
bf16 reciprocal of a *stale* (4-step-old) column sum — the scale is
recorded and undone exactly on the host, and using a stale value keeps the
reciprocal chain off the critical path.

The serial per-step chain (PE matmuls -> sem -> DVE multiply -> sem -> PE)
is ~590ns of mostly pipeline-drain/semaphore/PSUM-access latency and is the
throughput wall; the 8 batch rows per core run as TWO groups of 4 so the
smaller DVE multiply (142ns vs 158ns) shortens that chain, with all other
work (emissions, column sums, renorm plumbing) scheduled into its idle
windows.

Emissions: em[b,t,h] = 0.25 * sum_s x[s,h,obs[b,t,s]] - L[h].  The host
pre-transposes x to a (S*V, H) bf16 row table; the device gathers rows
with indirect DMA (128 rows = 16 timesteps x 8 batch) one source at a
time, then sums the 4 sources AND transposes to H-major in one shot using
PSUM-accumulating identity matmuls on the TensorEngine, and applies
exp(0.25*x - L - kappa) on the ScalarEngine into the E-strip consumed by
the scan.  Block 0's E-strip is precomputed on the host to skip the
startup gather latency.

Sharding: data-parallel over batch (8 of 64 rows per core).  Tables are
replicated.  No collectives.
"""
import sys

sys.path.insert(0, "/opt/trn_rl_repo")

import numpy as np
import ml_dtypes

import concourse.bass as bass
import concourse.bacc as bacc
import concourse.tile as tile
import concourse.mybir as mybir
import concourse.bass_utils as bass_utils
from concourse.masks import make_identity

B, T, S, H, V = 64, 512, 4, 512, 10000
NC = 8            # cores
BL = B // NC      # batch rows per core
G = 2             # staggered scan groups per core
BG = BL // G      # batch rows per group
P_ = 128          # partitions
HCN = H // P_     # h chunks
TBLK = 16         # timesteps per gather block
RN = 16           # renorm interval
STALE = 4         # renorm uses colsum of phat_{t-STALE}
F32 = mybir.dt.float32
BF16 = mybir.dt.bfloat16
I32 = mybir.dt.int32
EXP = mybir.ActivationFunctionType.Exp
MULT = mybir.AluOpType.mult

_compiled = {}
LAST_T = T


def _renorm_steps(t_steps):
    return [t for t in range(1, t_steps) if t % RN == 0]


def build(t_steps=T):
    """Build + bacc-compile the per-core Bass program (identical on all cores)."""
    nblk = -(-t_steps // TBLK)
    renorms = _renorm_steps(t_steps)
    nrn = max(1, len(renorms))
    nc = bacc.Bacc("TRN2", target_bir_lowering=False, debug=False,
                   enable_asserts=False, num_devices=NC)

    tabt = nc.dram_tensor("tabt", [S * V, H], BF16, kind="ExternalInput").ap()
    pm_d = nc.dram_tensor("pm", [P_, HCN * HCN * P_], BF16, kind="ExternalInput").ap()
    idx_d = nc.dram_tensor("idx", [P_, S * nblk], I32, kind="ExternalInput").ap()
    bias_d = nc.dram_tensor("bias", [P_, HCN], F32, kind="ExternalInput").ap()
    eb0_d = nc.dram_tensor("eb0", [P_, 2 * TBLK * HCN * BL], BF16,
                           kind="ExternalInput").ap()
    rstrip_d = nc.dram_tensor("rstrip", [1, nblk * TBLK * BL], F32,
                              kind="ExternalOutput").ap()
    rinv_d = nc.dram_tensor("rinvstrip", [1, nrn * BL], F32,
                            kind="ExternalOutput").ap()

    with tile.TileContext(nc) as tc:
        with (tc.tile_pool(name="const", bufs=1) as cp,
              tc.tile_pool(name="estrip", bufs=3) as ep,
              tc.tile_pool(name="gath", bufs=10) as gp,
              tc.tile_pool(name="phat", bufs=3 * G) as pp,
              tc.tile_pool(name="small", bufs=4) as sp,
              tc.tile_pool(name="qpsum", bufs=1, space="PSUM") as qp,
              tc.tile_pool(name="rspsum", bufs=2, space="PSUM") as rsp,
              tc.tile_pool(name="tpsum", bufs=2, space="PSUM") as tp_,
              tc.tile_pool(name="ipsum", bufs=1, space="PSUM") as ip):

            # ---- constants (eb0+pm first, in parallel on two queues:
            # they gate the first scan step; idx/bias only gate gathers) ----
            pm_t = cp.tile([P_, HCN * HCN * P_], BF16, name="pmt")
            nc.scalar.dma_start(pm_t[:, :], pm_d[:, :])
            idx_t = cp.tile([P_, S * nblk], I32, name="idxt")
            nc.sync.dma_start(idx_t[:, :], idx_d[:, :])
            bias_t = cp.tile([P_, HCN], F32, name="biast")
            nc.scalar.dma_start(bias_t[:, :], bias_d[:, :])
            eb0_t = cp.tile([P_, 2 * TBLK * HCN * BL], BF16, name="eb0t")
            nc.sync.dma_start(eb0_t[:, :], eb0_d[:, :])
            ones128 = cp.tile([P_, 1], BF16, name="ones128")
            nc.gpsimd.memset(ones128[:, :], 1.0)
            onesrow = cp.tile([1, P_], BF16, name="onesrow")
            nc.gpsimd.memset(onesrow[:, :], 1.0)
            ident = cp.tile([P_, P_], BF16, name="ident")
            make_identity(nc, ident[:, :])
            rstrip_t = cp.tile([1, nblk * TBLK * BL], F32, name="rstript")
            rinv_t = cp.tile([1, nrn * BL], F32, name="rinvt")

            eb_list = [None] * nblk
            eb_list[0] = eb0_t[:, :TBLK * HCN * BL]
            if nblk > 1:
                eb_list[1] = eb0_t[:, TBLK * HCN * BL:]

            g_tiles = {}     # blk -> [4 gather tiles]
            tp_tiles = {}    # (blk, c) -> transpose psum tile
            rs_tiles = {}    # blk -> rstrip psum tile
            rinv_sb = {}     # (t, g) -> bf16 reciprocal staging tile
            rinv_ps = {}     # (t, g) -> [128,16] broadcast psum tile

            def emit_gather(blk, s):
                g = gp.tile([P_, H], BF16, tag="g", name=f"g{blk}_{s}")
                col = s * nblk + blk
                nc.gpsimd.indirect_dma_start(
                    out=g[:, :], out_offset=None, in_=tabt[:, :],
                    in_offset=bass.IndirectOffsetOnAxis(
                        ap=idx_t[:, col:col + 1], axis=0))
                g_tiles.setdefault(blk, [None] * S)[s] = g

            def emit_transpose(blk, c, s):
                if s == 0:
                    tp_tiles[(blk, c)] = tp_.tile([P_, P_], F32, tag="tp", name=f"tp{blk}_{c}")
                nc.tensor.matmul(tp_tiles[(blk, c)][:, :],
                                 lhsT=g_tiles[blk][s][:, c * P_:(c + 1) * P_],
                                 rhs=ident[:, :],
                                 start=(s == 0), stop=(s == S - 1))

            def emit_act(blk, c):
                if c == 0:
                    eb_list[blk] = ep.tile([P_, TBLK * HCN * BL], BF16,
                                           tag="eb", name=f"eb{blk}")
                eb4 = eb_list[blk].rearrange("p (t c b) -> p t c b",
                                             t=TBLK, c=HCN)
                tpp = tp_tiles.pop((blk, c))
                nc.scalar.activation(
                    eb4[:, :, c, :],
                    tpp.rearrange("p (t b) -> p t b", t=TBLK),
                    EXP, bias=bias_t[:, c:c + 1], scale=0.25)

            def rs_tile(m):
                if m not in rs_tiles:
                    rs_tiles[m] = rsp.tile([1, TBLK * BL], F32, tag="rs", name=f"rs{m}")
                return rs_tiles[m]

            def emit_rgroup(t_src, phat_jc, g):
                """Column sums of phat_{t_src} (group g) -> psum slot t_src."""
                rt = rs_tile(t_src // TBLK)
                slot = t_src % TBLK
                cs = slice(slot * BL + g * BG, slot * BL + (g + 1) * BG)
                for jc in range(HCN):
                    nc.tensor.matmul(rt[:, cs], lhsT=ones128[:, :],
                                     rhs=phat_jc[jc],
                                     start=(jc == 0), stop=(jc == HCN - 1))

            def emit_rs_copy(m):
                """PSUM rstrip block -> SBUF strip -> DRAM (streamed)."""
                rt = rs_tiles.pop(m)
                cs = slice(m * TBLK * BL, (m + 1) * TBLK * BL)
                nc.scalar.copy(rstrip_t[:, cs], rt[:, :])
                nc.sync.dma_start(rstrip_d[:, cs], rstrip_t[:, cs])

            # blocks 0 and 1 come precomputed from the host; prefetch
            # gathers for blocks 2 and 3 before the scan starts
            for m in (2, 3):
                if m < nblk:
                    for s in range(S):
                        emit_gather(m, s)

            eb0_4 = eb_list[0].rearrange("p (t c b) -> p t c b",
                                         t=TBLK, c=HCN)
            # phat[g] is a list of per-jc [128, BG] APs (matmul rhs slices)
            phat = [[eb0_4[:, 0, jc, g * BG:(g + 1) * BG]
                     for jc in range(HCN)] for g in range(G)]

            ridx_of = {t: i for i, t in enumerate(renorms)}

            for t in range(1, t_steps):
                blk, ph = t // TBLK, t % TBLK

                # ---- emission pipeline bookkeeping for this phase ----
                if ph % 2 == 0 and ph < 2 * S and blk >= 1 and blk + 3 < nblk:
                    emit_gather(blk + 3, ph // 2)
                if blk >= 1 and blk + 1 < nblk and ph < S * HCN // 2:
                    for k in range(2):
                        i = ph * 2 + k
                        c, s = i // S, i % S
                        emit_transpose(blk + 1, c, s)
                        if s == S - 1:
                            emit_act(blk + 1, c)

                # ---- scan step: both groups' matmul bundles first ----
                qs = []
                for g in range(G):
                    q = qp.tile([P_, HCN * BG], F32, tag=f"q{g}", name=f"q{t}_{g}")
                    for kc in range(HCN):
                        for jc in range(HCN):
                            nc.tensor.matmul(
                                q[:, kc * BG:(kc + 1) * BG],
                                lhsT=pm_t[:, (jc * HCN + kc) * P_:
                                          (jc * HCN + kc + 1) * P_],
                                rhs=phat[g][jc],
                                start=(jc == 0), stop=(jc == HCN - 1))
                    qs.append(q)
                if t + 2 in ridx_of:
                    # broadcast bf16 rinv (from stale colsum) to [128,16];
                    # applied to the E-strip slice of step t+2 below
                    for g in range(G):
                        rps = ip.tile([P_, HCN * BG], F32, tag=f"rp{g}",
                                      name=f"rp{t}_{g}")
                        rb = rinv_sb[(t + 2, g)]
                        for c in range(HCN):
                            nc.tensor.matmul(rps[:, c * BG:(c + 1) * BG],
                                             lhsT=onesrow[:, :],
                                             rhs=rb[:, :],
                                             start=True, stop=True)
                        rinv_ps[(t + 2, g)] = rps
                # column sums of the previous step's phat (same dep as mms,
                # ordered after them so they don't delay the q semaphores)
                for g in range(G):
                    emit_rgroup(t - 1, phat[g], g)

                for g in range(G):
                    tn2 = t + 3
                    if tn2 in ridx_of and tn2 < t_steps:
                        # colsum of phat_{tn2-STALE} lives in slot tn2-STALE
                        src_t = rs_tile((tn2 - STALE) // TBLK)
                        slot = (tn2 - STALE) % TBLK
                        cs = slice(slot * BL + g * BG,
                                   slot * BL + (g + 1) * BG)
                        rb = sp.tile([1, BG], BF16, tag=f"rb{g}", name=f"rb{t}_{g}")
                        with nc.allow_low_precision(
                                reason="renorm scale recorded+undone exactly"):
                            nc.vector.reciprocal(rb[:, :], src_t[:, cs])
                        rinv_sb[(tn2, g)] = rb
                        ri = ridx_of[tn2]
                        nc.scalar.copy(
                            rinv_t[:, ri * BL + g * BG:
                                   ri * BL + (g + 1) * BG], rb[:, :])

                ebt4 = eb_list[blk].rearrange("p (t c b) -> p t c b",
                                              t=TBLK, c=HCN)
                for g in range(G):
                    pnew = pp.tile([P_, HCN * BG], BF16, tag=f"ph{g}", name=f"pn{t}_{g}")
                    pnew3 = pnew.rearrange("p (c b) -> p c b", c=HCN)
                    q3 = qs[g].rearrange("p (c b) -> p c b", c=HCN)
                    nc.vector.tensor_tensor(
                        pnew3[:, :, :], q3[:, :, :],
                        ebt4[:, ph, :, g * BG:(g + 1) * BG], MULT)
                    phat[g] = [pnew[:, jc * BG:(jc + 1) * BG]
                               for jc in range(HCN)]

                # off-critical-path renorm plumbing (stale by >= 2 steps)
                for g in range(G):
                    tn = t + 2 if g == 0 else t + 1
                    if tn in ridx_of and tn < t_steps:
                        # pre-scale the renorm step's E-strip slice by the
                        # stale reciprocal (SBUF multiply, off the scan loop);
                        # group 0 two steps ahead, group 1 one step ahead
                        rps = rinv_ps.pop((tn, g))
                        rps3 = rps.rearrange("p (c b) -> p c b", c=HCN)
                        nblk4 = eb_list[tn // TBLK].rearrange(
                            "p (t c b) -> p t c b", t=TBLK, c=HCN)
                        nc.vector.tensor_tensor(
                            nblk4[:, tn % TBLK, :, g * BG:(g + 1) * BG],
                            nblk4[:, tn % TBLK, :, g * BG:(g + 1) * BG],
                            rps3[:, :, :], MULT)

                # block m's psum strip complete once slot 15 written (t-1)
                if ph == 0 and blk >= 1:
                    emit_rs_copy(blk - 1)

            # ---- tail: colsums of the final phat, last block copy, DMA out
            for g in range(G):
                emit_rgroup(t_steps - 1, phat[g], g)
            emit_rs_copy(t_steps // TBLK - 1 if t_steps % TBLK == 0
                         else t_steps // TBLK)
            nc.sync.dma_start(rinv_d[:, :], rinv_t[:, :])

    nc.compile()
    return nc


def _get_compiled(t_steps=T):
    if t_steps not in _compiled:
        _compiled[t_steps] = build(t_steps)
    return _compiled[t_steps]


def _host_prep(obs, emis, tran, priors, t_steps):
    """Returns (shared_inputs, per_core_inputs, kappa)."""
    nblk = -(-t_steps // TBLK)
    tpad = nblk * TBLK
    if tpad > obs.shape[1]:
        obs = np.concatenate(
            [obs, np.repeat(obs[:, -1:, :], tpad - obs.shape[1], axis=1)],
            axis=1)
    # transition softmax -> bf16 chunk layout [j, (jc*HCN+kc)*128 + k]
    m = tran.max(axis=1, keepdims=True)
    e = np.exp(tran - m, dtype=np.float32)
    P = (e / e.sum(axis=1, keepdims=True)).astype(ml_dtypes.bfloat16)
    pm = np.ascontiguousarray(
        P.reshape(HCN, P_, HCN, P_).transpose(1, 0, 2, 3).reshape(P_, -1))

    # transposed bf16 emission table, rows indexed by s*V+v
    tabT = np.ascontiguousarray(
        emis.transpose(0, 2, 1)).astype(ml_dtypes.bfloat16).reshape(S * V, H)

    # L[h] and kappa
    mx = emis.max(axis=2)                                   # (S,H)
    lse = mx + np.log(np.exp(emis - mx[:, :, None],
                             dtype=np.float32).sum(axis=2))
    L = 0.25 * lse.sum(axis=0)                              # (H,)
    kap_h = 0.25 * mx.sum(axis=0) - L
    kappa = float(kap_h.max())
    bias = np.ascontiguousarray(
        (-(L + kappa)).astype(np.float32).reshape(HCN, P_).T)   # (128,4)
    expp = np.exp(priors, dtype=np.float32)                     # (H,)

    # per-core gather row indices: idx[p=(tt*BL+bb), s*nblk+blk]
    per_core = []
    svec = (np.arange(S, dtype=np.int64) * V)
    tabT_f = tabT.astype(np.float32).reshape(S, V, H)
    for c in range(NC):
        o = obs[c * BL:(c + 1) * BL, :tpad, :]              # (BL,t,S)
        oi = o + svec[None, None, :]
        oi = oi.transpose(1, 0, 2)                          # (t, BL, S)
        oi = oi.reshape(nblk, TBLK, BL, S)
        oi = oi.transpose(1, 2, 3, 0).reshape(TBLK * BL, S * nblk)
        idx = np.ascontiguousarray(oi.astype(np.int32))

        # host-computed E-strip for blocks 0-1: layout [128, (blk, t, c, b)]
        o0 = o[:, :2 * TBLK, :]                             # (BL,32,S)
        x0 = tabT_f[np.arange(S)[None, None, :], o0].sum(axis=2)  # (BL,32,H)
        e0 = np.exp(0.25 * x0 - (L + kappa)[None, None, :])       # (BL,32,H)
        e0[:, 0, :] *= expp[None, :]
        parts = []
        for m in range(2):
            eb = e0[:, m * TBLK:(m + 1) * TBLK].transpose(2, 1, 0)  # (H,16,BL)
            eb = eb.reshape(HCN, P_, TBLK, BL).transpose(1, 2, 0, 3)
            parts.append(eb.reshape(P_, TBLK * HCN * BL))
        eb0 = np.ascontiguousarray(
            np.concatenate(parts, axis=1)).astype(ml_dtypes.bfloat16)
        per_core.append({"idx": idx, "eb0": eb0})

    shared = {"tabt": tabT, "pm": pm, "bias": bias}
    return shared, per_core, kappa


def _host_post(results, lengths, kappa, t_steps):
    renorms = _renorm_steps(t_steps)
    nrn = max(1, len(renorms))
    ans = np.zeros((B, 1), np.float32)
    tt = np.arange(t_steps, dtype=np.float64)
    for c in range(NC):
        r = np.asarray(results[c]["rstrip"]).reshape(-1, BL)[
            :t_steps].astype(np.float64)
        rinv = np.asarray(results[c]["rinvstrip"]).astype(
            np.float64).reshape(nrn, BL)
        rho_log = np.zeros((t_steps, BL), np.float64)
        for k, t in enumerate(renorms):
            rho_log[t] = np.log(rinv[k])
        logsums = np.log(r) + (tt[:, None] + 1.0) * kappa \
            - np.cumsum(rho_log, axis=0)
        lens = np.clip(lengths[c * BL:(c + 1) * BL], 1, t_steps)
        ans[c * BL:(c + 1) * BL, 0] = logsums[
            lens - 1, np.arange(BL)].astype(np.float32)
    return ans


def run(inputs, t_steps=None, trace=False):
    obs = np.asarray(inputs["obs"])
    lengths = np.asarray(inputs["lengths"])
    emis = np.asarray(inputs["unnormalized_emis"], np.float32)
    tran = np.asarray(inputs["unnormalized_tran"], np.float32)
    priors = np.asarray(inputs["log_state_priors"], np.float32)
    if t_steps is None:
        # the scan only needs to reach the longest sequence
        t_steps = int(min(T, max(48, int(lengths.max()))))
    global LAST_T
    LAST_T = t_steps

    nc = _get_compiled(t_steps)
    shared, per_core, kappa = _host_prep(obs, emis, tran, priors, t_steps)
    in_maps = [dict(shared, **per_core[c]) for c in range(NC)]
    res = bass_utils.run_bass_kernel_spmd(nc, in_maps,
                                          core_ids=list(range(NC)),
                                          trace=trace)
    ans = _host_post(res.results, lengths, kappa, t_steps)
    return ans, res


def kernel(obs, lengths, unnormalized_emis, unnormalized_tran,
           log_state_priors):
    ans, _ = run(dict(obs=obs, lengths=lengths,
                      unnormalized_emis=unnormalized_emis,
                      unnormalized_tran=unnormalized_tran,
                      log_state_priors=log_state_priors))
    return ans


# revision 29
# speedup vs baseline: 1.0191x; 1.0004x over previous
"""Trainium2 Bass kernel for the HMM forward-algorithm problem.

Strategy
--------
The reference does, per time step, a log-domain matrix-vector product
  alpha_t[b,k] = em[b,t,k] + logsumexp_j(alpha_{t-1}[b,j] + tran[j,k])
followed by logsumexp_k.  We run the whole recurrence in *probability*
domain on the TensorEngine:

  phat_t = E_t  *  (phat_{t-1} @ P)          (elementwise * matmul)

where P = softmax(tran) rows (constant) and E_t = exp(em_t - kappa) with a
global shift kappa that keeps E <= ~1.  The per-step logsumexp_k output
reduces to log(sum_k phat_t) + known offsets; column sums are accumulated
on the TensorEngine with a ones-vector matmul into a per-block PSUM strip.
phat decays by ~e^-3 per step, so every RN=16 steps it is rescaled by the
bf16 reciprocal of a *stale* (4-step-old) column sum — the scale is
recorded and undone exactly on the host, and using a stale value keeps the
reciprocal chain off the critical path.

The serial per-step chain (PE matmuls -> sem -> DVE multiply -> sem -> PE)
is ~590ns of mostly pipeline-drain/semaphore/PSUM-access latency and is the
throughput wall; the 8 batch rows per core run as TWO groups of 4 so the
smaller DVE multiply (142ns vs 158ns) shortens that chain, with all other
work (emissions, column sums, renorm plumbing) scheduled into its idle
windows.

Emissions: em[b,t,h] = 0.25 * sum_s x[s,h,obs[b,t,s]] - L[h].  The host
pre-transposes x to a (S*V, H) bf16 row table; the device gathers rows
with indirect DMA (128 rows = 16 timesteps x 8 batch) one source at a
time, then sums the 4 sources AND transposes to H-major in one shot using
PSUM-accumulating identity matmuls on the TensorEngine, and applies
exp(0.25*x - L - kappa) on the ScalarEngine into the E-strip consumed by
the scan.  Block 0's E-strip is precomputed on the host to skip the
startup gather latency.

Sharding: data-parallel over batch (8 of 64 rows per core).  Tables are
replicated.  No collectives.
"""
import sys

sys.path.insert(0, "/opt/trn_rl_repo")

import numpy as np
import ml_dtypes

import concourse.bass as bass
import concourse.bacc as bacc
import concourse.tile as tile
import concourse.mybir as mybir
import concourse.bass_utils as bass_utils
from concourse.masks import make_identity

B, T, S, H, V = 64, 512, 4, 512, 10000
NC = 8            # cores
BL = B // NC      # batch rows per core
G = 2             # staggered scan groups per core
BG = BL // G      # batch rows per group
P_ = 128          # partitions
HCN = H // P_     # h chunks
TBLK = 16         # timesteps per gather block
RN = 16           # renorm interval
STALE = 4         # renorm uses colsum of phat_{t-STALE}
F32 = mybir.dt.float32
BF16 = mybir.dt.bfloat16
I32 = mybir.dt.int32
EXP = mybir.ActivationFunctionType.Exp
MULT = mybir.AluOpType.mult

_compiled = {}
LAST_T = T


def _renorm_steps(t_steps):
    return [t for t in range(1, t_steps) if t % RN == 0]


def build(t_steps=T):
    """Build + bacc-compile the per-core Bass program (identical on all cores)."""
    nblk = -(-t_steps // TBLK)
    renorms = _renorm_steps(t_steps)
    nrn = max(1, len(renorms))
    nc = bacc.Bacc("TRN2", target_bir_lowering=False, debug=False,
                   enable_asserts=False, num_devices=NC)

    tabt = nc.dram_tensor("tabt", [S * V, H], BF16, kind="ExternalInput").ap()
    pm_d = nc.dram_tensor("pm", [P_, HCN * HCN * P_], BF16, kind="ExternalInput").ap()
    idx_d = nc.dram_tensor("idx", [P_, S * nblk], I32, kind="ExternalInput").ap()
    bias_d = nc.dram_tensor("bias", [P_, HCN], F32, kind="ExternalInput").ap()
    eb0_d = nc.dram_tensor("eb0", [P_, 2 * TBLK * HCN * BL], BF16,
                           kind="ExternalInput").ap()
    rstrip_d = nc.dram_tensor("rstrip", [1, nblk * TBLK * BL], F32,
                              kind="ExternalOutput").ap()
    rinv_d = nc.dram_tensor("rinvstrip", [1, nrn * BL], F32,
                            kind="ExternalOutput").ap()

    with tile.TileContext(nc) as tc:
        with (tc.tile_pool(name="const", bufs=1) as cp,
              tc.tile_pool(name="estrip", bufs=3) as ep,
              tc.tile_pool(name="gath", bufs=10) as gp,
              tc.tile_pool(name="phat", bufs=3 * G) as pp,
              tc.tile_pool(name="small", bufs=4) as sp,
              tc.tile_pool(name="qpsum", bufs=1, space="PSUM") as qp,
              tc.tile_pool(name="rspsum", bufs=2, space="PSUM") as rsp,
              tc.tile_pool(name="tpsum", bufs=2, space="PSUM") as tp_,
              tc.tile_pool(name="ipsum", bufs=1, space="PSUM") as ip):

            # ---- constants (eb0+pm first, in parallel on two queues:
            # they gate the first scan step; idx/bias only gate gathers) ----
            eb0_t = cp.tile([P_, 2 * TBLK * HCN * BL], BF16, name="eb0t")
            nc.sync.dma_start(eb0_t[:, :], eb0_d[:, :])
            pm_t = cp.tile([P_, HCN * HCN * P_], BF16, name="pmt")
            nc.scalar.dma_start(pm_t[:, :], pm_d[:, :])
            idx_t = cp.tile([P_, S * nblk], I32, name="idxt")
            nc.sync.dma_start(idx_t[:, :], idx_d[:, :])
            bias_t = cp.tile([P_, HCN], F32, name="biast")
            nc.scalar.dma_start(bias_t[:, :], bias_d[:, :])
            ones128 = cp.tile([P_, 1], BF16, name="ones128")
            nc.gpsimd.memset(ones128[:, :], 1.0)
            onesrow = cp.tile([1, P_], BF16, name="onesrow")
            nc.gpsimd.memset(onesrow[:, :], 1.0)
            ident = cp.tile([P_, P_], BF16, name="ident")
            make_identity(nc, ident[:, :])
            rstrip_t = cp.tile([1, nblk * TBLK * BL], F32, name="rstript")
            rinv_t = cp.tile([1, nrn * BL], F32, name="rinvt")

            eb_list = [None] * nblk
            eb_list[0] = eb0_t[:, :TBLK * HCN * BL]
            if nblk > 1:
                eb_list[1] = eb0_t[:, TBLK * HCN * BL:]

            g_tiles = {}     # blk -> [4 gather tiles]
            tp_tiles = {}    # (blk, c) -> transpose psum tile
            rs_tiles = {}    # blk -> rstrip psum tile
            rinv_sb = {}     # (t, g) -> bf16 reciprocal staging tile
            rinv_ps = {}     # (t, g) -> [128,16] broadcast psum tile

            def emit_gather(blk, s):
                g = gp.tile([P_, H], BF16, tag="g", name=f"g{blk}_{s}")
                col = s * nblk + blk
                nc.gpsimd.indirect_dma_start(
                    out=g[:, :], out_offset=None, in_=tabt[:, :],
                    in_offset=bass.IndirectOffsetOnAxis(
                        ap=idx_t[:, col:col + 1], axis=0))
                g_tiles.setdefault(blk, [None] * S)[s] = g

            def emit_transpose(blk, c, s):
                if s == 0:
                    tp_tiles[(blk, c)] = tp_.tile([P_, P_], F32, tag="tp", name=f"tp{blk}_{c}")
                nc.tensor.matmul(tp_tiles[(blk, c)][:, :],
                                 lhsT=g_tiles[blk][s][:, c * P_:(c + 1) * P_],
                                 rhs=ident[:, :],
                                 start=(s == 0), stop=(s == S - 1))

            def emit_act(blk, c):
                if c == 0:
                    eb_list[blk] = ep.tile([P_, TBLK * HCN * BL], BF16,
                                           tag="eb", name=f"eb{blk}")
                eb4 = eb_list[blk].rearrange("p (t c b) -> p t c b",
                                             t=TBLK, c=HCN)
                tpp = tp_tiles.pop((blk, c))
                nc.scalar.activation(
                    eb4[:, :, c, :],
                    tpp.rearrange("p (t b) -> p t b", t=TBLK),
                    EXP, bias=bias_t[:, c:c + 1], scale=0.25)

            def rs_tile(m):
                if m not in rs_tiles:
                    rs_tiles[m] = rsp.tile([1, TBLK * BL], F32, tag="rs", name=f"rs{m}")
                return rs_tiles[m]

            def emit_rgroup(t_src, phat_jc, g):
                """Column sums of phat_{t_src} (group g) -> psum slot t_src."""
                rt = rs_tile(t_src // TBLK)
                slot = t_src % TBLK
                cs = slice(slot * BL + g * BG, slot * BL + (g + 1) * BG)
                for jc in range(HCN):
                    nc.tensor.matmul(rt[:, cs], lhsT=ones128[:, :],
                                     rhs=phat_jc[jc],
                                     start=(jc == 0), stop=(jc == HCN - 1))

            def emit_rs_copy(m):
                """PSUM rstrip block -> SBUF strip -> DRAM (streamed)."""
                rt = rs_tiles.pop(m)
                cs = slice(m * TBLK * BL, (m + 1) * TBLK * BL)
                nc.scalar.copy(rstrip_t[:, cs], rt[:, :])
                nc.sync.dma_start(rstrip_d[:, cs], rstrip_t[:, cs])

            # blocks 0 and 1 come precomputed from the host; prefetch
            # gathers for blocks 2 and 3 before the scan starts
            for m in (2, 3):
                if m < nblk:
                    for s in range(S):
                        emit_gather(m, s)

            eb0_4 = eb_list[0].rearrange("p (t c b) -> p t c b",
                                         t=TBLK, c=HCN)
            # phat[g] is a list of per-jc [128, BG] APs (matmul rhs slices)
            phat = [[eb0_4[:, 0, jc, g * BG:(g + 1) * BG]
                     for jc in range(HCN)] for g in range(G)]

            ridx_of = {t: i for i, t in enumerate(renorms)}

            for t in range(1, t_steps):
                blk, ph = t // TBLK, t % TBLK

                # ---- emission pipeline bookkeeping for this phase ----
                if ph % 2 == 0 and ph < 2 * S and blk >= 1 and blk + 3 < nblk:
                    emit_gather(blk + 3, ph // 2)
                if blk >= 1 and blk + 1 < nblk and ph < S * HCN // 2:
                    for k in range(2):
                        i = ph * 2 + k
                        c, s = i // S, i % S
                        emit_transpose(blk + 1, c, s)
                        if s == S - 1:
                            emit_act(blk + 1, c)

                # ---- scan step: both groups' matmul bundles first ----
                qs = []
                for g in range(G):
                    q = qp.tile([P_, HCN * BG], F32, tag=f"q{g}", name=f"q{t}_{g}")
                    for kc in range(HCN):
                        for jc in range(HCN):
                            nc.tensor.matmul(
                                q[:, kc * BG:(kc + 1) * BG],
                                lhsT=pm_t[:, (jc * HCN + kc) * P_:
                                          (jc * HCN + kc + 1) * P_],
                                rhs=phat[g][jc],
                                start=(jc == 0), stop=(jc == HCN - 1))
                    qs.append(q)
                if t + 2 in ridx_of:
                    # broadcast bf16 rinv (from stale colsum) to [128,16];
                    # applied to the E-strip slice of step t+2 below
                    for g in range(G):
                        rps = ip.tile([P_, HCN * BG], F32, tag=f"rp{g}",
                                      name=f"rp{t}_{g}")
                        rb = rinv_sb[(t + 2, g)]
                        for c in range(HCN):
                            nc.tensor.matmul(rps[:, c * BG:(c + 1) * BG],
                                             lhsT=onesrow[:, :],
                                             rhs=rb[:, :],
                                             start=True, stop=True)
                        rinv_ps[(t + 2, g)] = rps
                # column sums of the previous step's phat (same dep as mms,
                # ordered after them so they don't delay the q semaphores)
                for g in range(G):
                    emit_rgroup(t - 1, phat[g], g)

                for g in range(G):
                    tn2 = t + 3
                    if tn2 in ridx_of and tn2 < t_steps:
                        # colsum of phat_{tn2-STALE} lives in slot tn2-STALE
                        src_t = rs_tile((tn2 - STALE) // TBLK)
                        slot = (tn2 - STALE) % TBLK
                        cs = slice(slot * BL + g * BG,
                                   slot * BL + (g + 1) * BG)
                        rb = sp.tile([1, BG], BF16, tag=f"rb{g}", name=f"rb{t}_{g}")
                        with nc.allow_low_precision(
                                reason="renorm scale recorded+undone exactly"):
                            nc.vector.reciprocal(rb[:, :], src_t[:, cs])
                        rinv_sb[(tn2, g)] = rb
                        ri = ridx_of[tn2]
                        nc.scalar.copy(
                            rinv_t[:, ri * BL + g * BG:
                                   ri * BL + (g + 1) * BG], rb[:, :])

                ebt4 = eb_list[blk].rearrange("p (t c b) -> p t c b",
                                              t=TBLK, c=HCN)
                for g in range(G):
                    pnew = pp.tile([P_, HCN * BG], BF16, tag=f"ph{g}", name=f"pn{t}_{g}")
                    pnew3 = pnew.rearrange("p (c b) -> p c b", c=HCN)
                    q3 = qs[g].rearrange("p (c b) -> p c b", c=HCN)
                    nc.vector.tensor_tensor(
                        pnew3[:, :, :], q3[:, :, :],
                        ebt4[:, ph, :, g * BG:(g + 1) * BG], MULT)
                    phat[g] = [pnew[:, jc * BG:(jc + 1) * BG]
                               for jc in range(HCN)]

                # off-critical-path renorm plumbing (stale by >= 2 steps)
                for g in range(G):
                    tn = t + 2 if g == 0 else t + 1
                    if tn in ridx_of and tn < t_steps:
                        # pre-scale the renorm step's E-strip slice by the
                        # stale reciprocal (SBUF multiply, off the scan loop);
                        # group 0 two steps ahead, group 1 one step ahead
                        rps = rinv_ps.pop((tn, g))
                        rps3 = rps.rearrange("p (c b) -> p c b", c=HCN)
                        nblk4 = eb_list[tn // TBLK].rearrange(
                            "p (t c b) -> p t c b", t=TBLK, c=HCN)
                        nc.vector.tensor_tensor(
                            nblk4[:, tn % TBLK, :, g * BG:(g + 1) * BG],
                            nblk4[:, tn % TBLK, :, g * BG:(g + 1) * BG],
                            rps3[:, :, :], MULT)

                # block m's psum strip complete once slot 15 written (t-1)
                if ph == 0 and blk >= 1:
                    emit_rs_copy(blk - 1)

            # ---- tail: colsums of the final phat, last block copy, DMA out
            for g in range(G):
                emit_rgroup(t_steps - 1, phat[g], g)
            emit_rs_copy(t_steps // TBLK - 1 if t_steps % TBLK == 0
                         else t_steps // TBLK)
            nc.scalar.dma_start(rinv_d[:, :], rinv_t[:, :])

    nc.compile()
    return nc


def _get_compiled(t_steps=T):
    if t_steps not in _compiled:
        _compiled[t_steps] = build(t_steps)
    return _compiled[t_steps]


def _host_prep(obs, emis, tran, priors, t_steps):
    """Returns (shared_inputs, per_core_inputs, kappa)."""
    nblk = -(-t_steps // TBLK)
    tpad = nblk * TBLK
    if tpad > obs.shape[1]:
        obs = np.concatenate(
            [obs, np.repeat(obs[:, -1:, :], tpad - obs.shape[1], axis=1)],
            axis=1)
    # transition softmax -> bf16 chunk layout [j, (jc*HCN+kc)*128 + k]
    m = tran.max(axis=1, keepdims=True)
    e = np.exp(tran - m, dtype=np.float32)
    P = (e / e.sum(axis=1, keepdims=True)).astype(ml_dtypes.bfloat16)
    pm = np.ascontiguousarray(
        P.reshape(HCN, P_, HCN, P_).transpose(1, 0, 2, 3).reshape(P_, -1))

    # transposed bf16 emission table, rows indexed by s*V+v
    tabT = np.ascontiguousarray(
        emis.transpose(0, 2, 1)).astype(ml_dtypes.bfloat16).reshape(S * V, H)

    # L[h] and kappa
    mx = emis.max(axis=2)                                   # (S,H)
    lse = mx + np.log(np.exp(emis - mx[:, :, None],
                             dtype=np.float32).sum(axis=2))
    L = 0.25 * lse.sum(axis=0)                              # (H,)
    kap_h = 0.25 * mx.sum(axis=0) - L
    kappa = float(kap_h.max())
    bias = np.ascontiguousarray(
        (-(L + kappa)).astype(np.float32).reshape(HCN, P_).T)   # (128,4)
    expp = np.exp(priors, dtype=np.float32)                     # (H,)

    # per-core gather row indices: idx[p=(tt*BL+bb), s*nblk+blk]
    per_core = []
    svec = (np.arange(S, dtype=np.int64) * V)
    tabT_f = tabT.astype(np.float32).reshape(S, V, H)
    for c in range(NC):
        o = obs[c * BL:(c + 1) * BL, :tpad, :]              # (BL,t,S)
        oi = o + svec[None, None, :]
        oi = oi.transpose(1, 0, 2)                          # (t, BL, S)
        oi = oi.reshape(nblk, TBLK, BL, S)
        oi = oi.transpose(1, 2, 3, 0).reshape(TBLK * BL, S * nblk)
        idx = np.ascontiguousarray(oi.astype(np.int32))

        # host-computed E-strip for blocks 0-1: layout [128, (blk, t, c, b)]
        o0 = o[:, :2 * TBLK, :]                             # (BL,32,S)
        x0 = tabT_f[np.arange(S)[None, None, :], o0].sum(axis=2)  # (BL,32,H)
        e0 = np.exp(0.25 * x0 - (L + kappa)[None, None, :])       # (BL,32,H)
        e0[:, 0, :] *= expp[None, :]
        parts = []
        for m in range(2):
            eb = e0[:, m * TBLK:(m + 1) * TBLK].transpose(2, 1, 0)  # (H,16,BL)
            eb = eb.reshape(HCN, P_, TBLK, BL).transpose(1, 2, 0, 3)
            parts.append(eb.reshape(P_, TBLK * HCN * BL))
        eb0 = np.ascontiguousarray(
            np.concatenate(parts, axis=1)).astype(ml_dtypes.bfloat16)
        per_core.append({"idx": idx, "eb0": eb0})

    shared = {"tabt": tabT, "pm": pm, "bias": bias}
    return shared, per_core, kappa


def _host_post(results, lengths, kappa, t_steps):
    renorms = _renorm_steps(t_steps)
    nrn = max(1, len(renorms))
    ans = np.zeros((B, 1), np.float32)
    tt = np.arange(t_steps, dtype=np.float64)
    for c in range(NC):
        r = np.asarray(results[c]["rstrip"]).reshape(-1, BL)[
            :t_steps].astype(np.float64)
        rinv = np.asarray(results[c]["rinvstrip"]).astype(
            np.float64).reshape(nrn, BL)
        rho_log = np.zeros((t_steps, BL), np.float64)
        for k, t in enumerate(renorms):
            rho_log[t] = np.log(rinv[k])
        logsums = np.log(r) + (tt[:, None] + 1.0) * kappa \
            - np.cumsum(rho_log, axis=0)
        lens = np.clip(lengths[c * BL:(c + 1) * BL], 1, t_steps)
        ans[c * BL:(c + 1) * BL, 0] = logsums[
            lens - 1, np.arange(BL)].astype(np.float32)
    return ans


def run(inputs, t_steps=None, trace=False):
    obs = np.asarray(inputs["obs"])
    lengths = np.asarray(inputs["lengths"])
    emis = np.asarray(inputs["unnormalized_emis"], np.float32)
    tran = np.asarray(inputs["unnormalized_tran"], np.float32)
    priors = np.asarray(inputs["log_state_priors"], np.float32)
    if t_steps is None:
        # the scan only needs to reach the longest sequence
        t_steps = int(min(T, max(48, int(lengths.max()))))
    global LAST_T
    LAST_T = t_steps

    nc = _get_compiled(t_steps)
    shared, per_core, kappa = _host_prep(obs, emis, tran, priors, t_steps)
    in_maps = [dict(shared, **per_core[c]) for c in range(NC)]
    res = bass_utils.run_bass_kernel_spmd(nc, in_maps,
                                          core_ids=list(range(NC)),
                                          trace=trace)
    ans = _host_post(res.results, lengths, kappa, t_steps)
    return ans, res


def kernel(obs, lengths, unnormalized_emis, unnormalized_tran,
           log_state_priors):
    ans, _ = run(dict(obs=obs, lengths=lengths,
                      unnormalized_emis=unnormalized_emis,
                      unnormalized_tran=unnormalized_tran,
                      log_state_priors=log_state_priors))
    return ans
